# revision 22
# baseline (speedup 1.0000x reference)
"""ConvCapsuleLayer TRN2 kernel v13.

Sharding: 8 cores = B(2) x D-quarters(4); zero cross-core communication.
Structure = v9 (serial blocks, PSUM-resident logits) - measured on HW the
device throttles to ~1.65 concurrently-active engines (activity monitor
util limit 0.5), so wall ~= 0.6 * sum(engine busy); pipelining/offload
don't help (v10 gpsimd offload and v11 3-stage software pipeline both
regressed), only total-engine-time cuts do.

v12 total-work cuts vs v9:
 - weighted_pre: ALL 4 ic route-bcasts consumed straight from PSUM via
   chunked muls (v9 copied ic2/3 to SBUF via ACT then full-width muls:
   3x589 ACT + 660 DVE = 2427ns vs 3x505 = 1515ns per ic-iter).
 - conv PSUM->SBUF copies on DVE tensor_copy (505ns) instead of ACT
   activation-copy (589ns).
 - squash t1 = 1 + s*nb on DVE tensor_scalar chunks (505ns) instead of
   ACT identity (589ns); also drops ACT table pressure.
All PE operands fp16 (fp32r/fp32 measured slower); PSUM accumulation
fp32.  Conv kd-MIXED packing: 27 taps baked into contraction as 3x K=128
+ 1x K=48 matmuls per 384-col chunk; shifts/plane-selection host-side.
"""
import sys
import numpy as np

sys.path.insert(0, "/opt/trn_rl_repo")

import concourse.bass as bass
import concourse.mybir as mybir
from concourse import bacc, tile
from contextlib import ExitStack

F32 = mybir.dt.float32
F16 = mybir.dt.float16
AF = mybir.ActivationFunctionType
ALU = mybir.AluOpType

B, D, H, W, IC, A = 2, 24, 48, 48, 4, 16
NC, NA = 8, 16
OC = 128
DPC = 6
DSLAB = DPC + 2
HP = 50
PLANE_POS = H * W      # 2304
CN = 384               # chunk cols (8 h-rows)
NCH = PLANE_POS // CN  # 6 chunks per plane
CROWS = CN // W        # 8
HPL = PLANE_POS // 2   # 1152, half-plane
NCH2 = HPL // CN       # 3 chunks per half-plane


def build_program():
    nc = bacc.Bacc("TRN2", target_bir_lowering=False, debug=False, num_devices=8)
    xg_e = nc.dram_tensor("xg", [DPC, IC, 3, OC, HP * HP], F16, kind="ExternalInput").ap()
    xgc_e = nc.dram_tensor("xgc", [DPC, IC, 48, HP * HP], F16, kind="ExternalInput").ap()
    wg_e = nc.dram_tensor("wg", [3, OC, OC], F16, kind="ExternalInput").ap()
    wgc_e = nc.dram_tensor("wgc", [OC, OC], F16, kind="ExternalInput").ap()
    bias_e = nc.dram_tensor("bias", [OC, 1], F32, kind="ExternalInput").ap()
    bias8_e = nc.dram_tensor("bias8", [OC, 1], F32, kind="ExternalInput").ap()
    masks_e = nc.dram_tensor("masks", [10, OC, OC], F16, kind="ExternalInput").ap()
    out_e = nc.dram_tensor("out", [DPC, OC, PLANE_POS], F16, kind="ExternalOutput").ap()

    with ExitStack() as ctx:
        tc = ctx.enter_context(tile.TileContext(nc))
        cpool = ctx.enter_context(tc.tile_pool(name="const", bufs=1))
        planep = ctx.enter_context(tc.tile_pool(name="planes", bufs=2))
        votesp = ctx.enter_context(tc.tile_pool(name="votes", bufs=2))
        s16 = ctx.enter_context(tc.tile_pool(name="s16", bufs=2))
        s32 = ctx.enter_context(tc.tile_pool(name="s32", bufs=2))
        ps_conv = ctx.enter_context(tc.tile_pool(name="ps_conv", bufs=2, space="PSUM"))
        ps_L = ctx.enter_context(tc.tile_pool(name="ps_L", bufs=3, space="PSUM"))
        ps_bc = ctx.enter_context(tc.tile_pool(name="ps_bc", bufs=3, space="PSUM"))

        # --- resident constants ---
        wg_s = cpool.tile([OC, 3, OC], F16, tag="wg")
        nc.sync.dma_start(out=wg_s[:], in_=wg_e.rearrange("k p m -> p k m"))
        wgc_s = cpool.tile([OC, OC], F16, tag="wgc")
        nc.sync.dma_start(out=wgc_s[:], in_=wgc_e[:])
        bias_s = cpool.tile([OC, 1], F32, tag="bias")
        nc.sync.dma_start(out=bias_s[:], in_=bias_e[:])
        bias8_s = cpool.tile([OC, 1], F32, tag="bias8")
        nc.sync.dma_start(out=bias8_s[:], in_=bias8_e[:])
        mk_s = cpool.tile([OC, 10, OC], F16, tag="masks")
        nc.sync.dma_start(out=mk_s[:], in_=masks_e.rearrange("k p m -> p k m"))
        nshift = cpool.tile([OC, 1], F32, tag="nshift")
        nc.vector.memset(nshift[:], -10.0)
        esbc = mk_s[:, 0, :]
        ena8bc = mk_s[:, 1, :]
        erbc = [mk_s[:, 2 + i, :] for i in range(IC)]
        edl = [mk_s[:, 6 + i, :] for i in range(IC)]

        def squash_fac(pre, nrm_scale, t1_scale):
            """pre [OC,HPL] f16 -> fac [OC,HPL] f16 (replicated), using
            nb = sum_na (pre)^2 per capsule via ena8bc MM.
            fac = sqrt(nrm_scale*nb) / (1 + t1_scale*nb)."""
            sq = s16.tile([OC, HPL], F16, tag="sq", bufs=3)
            nc.vector.tensor_mul(out=sq[:], in0=pre, in1=pre)
            nbs = []
            for c in range(NCH2):
                nb = ps_bc.tile([OC, CN], F32, tag="bc", name=f"nb{c}")
                nc.tensor.matmul(out=nb[:], lhsT=ena8bc, rhs=sq[:, c * CN:(c + 1) * CN],
                                 start=True, stop=True)
                nbs.append(nb)
            nrm = s16.tile([OC, HPL], F16, tag="nrm")
            for c in range(NCH2):
                nc.scalar.activation(out=nrm[:, c * CN:(c + 1) * CN], in_=nbs[c][:],
                                     func=AF.Sqrt, scale=nrm_scale)
            t1 = s32.tile([OC, HPL], F32, tag="t1")
            for c in range(NCH2):
                nc.scalar.activation(out=t1[:, c * CN:(c + 1) * CN], in_=nbs[c][:],
                                     func=AF.Identity, bias=1.0, scale=t1_scale)
            nc.vector.reciprocal_approx_fast(out=t1[:], in_=t1[:])
            fac = s16.tile([OC, HPL], F16, tag="fac")
            nc.vector.tensor_mul(out=fac[:], in0=nrm[:], in1=t1[:])
            return fac

        def softmax_r(Ls):
            """Ls: list of 3 PSUM logit banks [OC,CN] (L32 layout) ->
            r [OC,HPL] f16 (normalized route, L32 layout).
            exp computed f16 with a -10 logit shift (shift-invariant for
            softmax; measured logits span [-8, 18.5], per-pos max >= 0.3, so
            e <= e^8.5 and the f32 denom stays >= 6e-5 - no over/underflow)."""
            e = s16.tile([OC, HPL], F16, tag="e")
            for c in range(NCH2):
                nc.scalar.activation(out=e[:, c * CN:(c + 1) * CN], in_=Ls[c][:],
                                     func=AF.Exp, bias=nshift[:])
            rs = s32.tile([OC, HPL], F32, tag="rs")
            for c in range(NCH2):
                sb = ps_bc.tile([OC, CN], F32, tag="bc")
                nc.tensor.matmul(out=sb[:], lhsT=esbc, rhs=e[:, c * CN:(c + 1) * CN],
                                 start=True, stop=True)
                nc.vector.reciprocal_approx_fast(
                    out=rs[:, c * CN:(c + 1) * CN], in_=sb[:])
            r = s16.tile([OC, HPL], F16, tag="r")
            nc.vector.tensor_mul(out=r[:], in0=e[:], in1=rs[:])
            return r

        def weighted_pre(vch, r):
            """pre = sum_ic bcast_ic(r)*votes_ic + bias, [OC,HPL] f16.
            All 4 ic route-bcasts consumed straight from PSUM (chunk muls)."""
            ps = []
            for i in range(IC):
                p = s16.tile([OC, HPL], F16, tag=f"p{i}", name=f"p{i}")
                for c in range(NCH2):
                    rb = ps_bc.tile([OC, CN], F32, tag="bc", name=f"rb{i}{c}")
                    nc.tensor.matmul(out=rb[:], lhsT=erbc[i],
                                     rhs=r[:, c * CN:(c + 1) * CN],
                                     start=True, stop=True)
                    nc.vector.tensor_mul(out=p[:, c * CN:(c + 1) * CN],
                                         in0=vch[i][:, c * CN:(c + 1) * CN],
                                         in1=rb[:])
                ps.append(p)
            a01 = s16.tile([OC, HPL], F16, tag="a01")
            nc.vector.tensor_add(out=a01[:], in0=ps[0][:], in1=ps[1][:])
            a23 = s16.tile([OC, HPL], F16, tag="a23")
            nc.vector.tensor_add(out=a23[:], in0=ps[2][:], in1=ps[3][:])
            pre = s16.tile([OC, HPL], F16, tag="pre")
            nc.vector.scalar_tensor_tensor(out=pre[:], in0=a01[:],
                                           scalar=bias_s[:], in1=a23[:],
                                           op0=ALU.add, op1=ALU.add)
            return pre

        def delta_accum(vcat, hs, act, Ls, first):
            """L_c += per-ic sum_na votes*act (edl MMs, L32 layout).
            One wide DVE mul: d[ic|pos] = vcat[ic-strided] * act (stride-0
            ic-broadcast on in1)."""
            dcat = s16.tile([OC, IC * HPL], F16, tag="dcat", name="dcat")
            vi = bass.AP(vcat.tensor, vcat.offset + hs,
                         [list(vcat.ap[0]), [PLANE_POS, IC], [1, HPL]])
            ab = bass.AP(act.tensor, act.offset,
                         [list(act.ap[0]), [0, IC], [1, HPL]])
            do = bass.AP(dcat.tensor, dcat.offset,
                         [list(dcat.ap[0]), [HPL, IC], [1, HPL]])
            nc.vector.tensor_mul(out=do, in0=vi, in1=ab)
            for c in range(NCH2):
                for i in range(IC):
                    nc.tensor.matmul(out=Ls[c][:], lhsT=edl[i],
                                     rhs=dcat[:, i * HPL + c * CN:i * HPL + (c + 1) * CN],
                                     start=(first and i == 0), stop=(i == IC - 1))

        # ===================== main loop =====================
        for dp in range(DPC):
            vcat = votesp.tile([OC, IC * PLANE_POS], F16, tag="v")
            for n in range(IC):
                xt = planep.tile([OC, 3, HP * HP], F16, tag="xt")
                nc.sync.dma_start(out=xt[:], in_=xg_e[dp, n].rearrange("g p m -> p g m"))
                xc = planep.tile([48, HP * HP], F16, tag="xc")
                nc.sync.dma_start(out=xc[:], in_=xgc_e[dp, n])
                v = vcat[:, n * PLANE_POS:(n + 1) * PLANE_POS]
                for c in range(NCH):
                    h0 = c * CROWS
                    pc = ps_conv.tile([OC, CN], F32, tag="conv")
                    off = h0 * HP + 1
                    tA = xt[:]
                    for g in range(3):
                        rhsA = bass.AP(tA.tensor, tA.offset + g * HP * HP + off,
                                       [list(tA.ap[0]), [HP, CROWS], [1, 48]])
                        nc.tensor.matmul(out=pc[:], lhsT=wg_s[:, g, :],
                                         rhs=rhsA, start=(g == 0), stop=False)
                    tC = xc[0:48, :]
                    rhsC = bass.AP(tC.tensor, tC.offset + off,
                                   [list(tC.ap[0]), [HP, CROWS], [1, 48]])
                    nc.tensor.matmul(out=pc[:], lhsT=wgc_s[0:48, :],
                                     rhs=rhsC, start=False, stop=True)
                    nc.scalar.copy(out=v[:, c * CN:(c + 1) * CN], in_=pc[:])

            for h in range(2):
                hs = h * HPL
                vch = [vcat[:, n * PLANE_POS + hs:n * PLANE_POS + hs + HPL]
                       for n in range(IC)]
                # ---- iter 1: uniform route; P = sum votes + 8*bias ----
                # one wide TT computes [t01|t23] = [v0|v2] + [v1|v3]
                tt = s16.tile([OC, 2 * HPL], F16, tag="t0123", bufs=3)
                in0 = bass.AP(vcat.tensor, vcat.offset + hs,
                              [list(vcat.ap[0]), [2 * PLANE_POS, 2], [1, HPL]])
                in1 = bass.AP(vcat.tensor, vcat.offset + PLANE_POS + hs,
                              [list(vcat.ap[0]), [2 * PLANE_POS, 2], [1, HPL]])
                to = bass.AP(tt.tensor, tt.offset,
                             [list(tt.ap[0]), [HPL, 2], [1, HPL]])
                nc.vector.tensor_add(out=to, in0=in0, in1=in1)
                P = s16.tile([OC, HPL], F16, tag="P")
                nc.vector.scalar_tensor_tensor(out=P[:], in0=tt[:, 0:HPL],
                                               scalar=bias8_s[:], in1=tt[:, HPL:2 * HPL],
                                               op0=ALU.add, op1=ALU.add)
                # pre1_true = P/8: fold 1/64 into t1-scale, (1/8)*(1/8) into
                # nrm-scale (sqrt(nb/4096) = sqrt(n2)/8)
                fac1 = squash_fac(P[:], 1.0 / 4096, 1.0 / 64)
                act1 = s16.tile([OC, HPL], F16, tag="act")
                nc.vector.tensor_mul(out=act1[:], in0=P[:], in1=fac1[:])
                Ls = [ps_L.tile([OC, CN], F32, tag="L", name=f"L{c}")
                      for c in range(NCH2)]
                delta_accum(vcat, hs, act1[:], Ls, first=True)
                # ---- iter 2 ----
                r2 = softmax_r(Ls)
                pre2 = weighted_pre(vch, r2)
                fac2 = squash_fac(pre2[:], 1.0, 1.0)
                act2 = s16.tile([OC, HPL], F16, tag="act")
                nc.vector.tensor_mul(out=act2[:], in0=pre2[:], in1=fac2[:])
                delta_accum(vcat, hs, act2[:], Ls, first=False)
                # ---- iter 3 ----
                r3 = softmax_r(Ls)
                pre3 = weighted_pre(vch, r3)
                fac3 = squash_fac(pre3[:], 1.0, 1.0)
                o = s16.tile([OC, HPL], F16, tag="o")
                nc.vector.tensor_mul(out=o[:], in0=pre3[:], in1=fac3[:])
                nc.sync.dma_start(out=out_e[dp][:, hs:hs + HPL], in_=o[:])

    nc.compile()
    return nc


# ===================== host side =====================

def prep_inputs(x, conv_w, b):
    x = np.asarray(x, np.float32)
    conv_w = np.asarray(conv_w, np.float32)
    b = np.asarray(b, np.float32)

    wg = np.zeros((3, OC, OC), np.float32)
    wgc = np.zeros((OC, OC), np.float32)
    for t in range(27):
        kd, kh, kw = t // 9, (t % 9) // 3, t % 3
        blk = conv_w[:, :, kd, kh, kw].T  # [16(a), OC]
        if t < 24:
            wg[t // 8, 16 * (t % 8):16 * (t % 8) + 16] = blk
        else:
            wgc[16 * (t - 24):16 * (t - 24) + 16] = blk
    wg = wg.astype(np.float16)
    wgc = wgc.astype(np.float16)

    bias = b[0, 0, 0].reshape(OC, 1).astype(np.float32)
    bias8 = (8.0 * bias).astype(np.float32)

    # masks: [esbc, ena8bc, erbc0-3, edl0-3], each [OC(part) x OC(out)]
    masks = np.zeros((10, OC, OC), np.float32)
    for i in range(IC):
        for n in range(NC):
            for k in range(32):
                masks[0, 32 * i + n, 32 * i + k] = 1.0      # esbc
    for ncp in range(NC):
        for na in range(NA):
            for na2 in range(NA):
                masks[1, 16 * ncp + na, 16 * ncp + na2] = 1.0   # ena8bc
    for i in range(IC):
        for n in range(NC):
            for na in range(NA):
                masks[2 + i, 32 * i + n, 16 * n + na] = 1.0     # erbc_i
    for i in range(IC):
        for ncp in range(NC):
            for na in range(NA):
                for j in range(4):
                    masks[6 + i, 16 * ncp + na, 32 * i + 8 * j + ncp] = 1.0  # edl_i
    masks = masks.astype(np.float16)

    xt = np.transpose(x, (0, 4, 5, 1, 2, 3))  # [B, ICg, A, D, H, W]

    from numpy.lib.stride_tricks import sliding_window_view

    in_maps = []
    for core in range(8):
        bc, dq = core // 4, core % 4
        d0 = dq * DPC - 1
        xg = np.zeros((DPC, IC, 3, OC, HP * HP), np.float16)
        xgc = np.zeros((DPC, IC, 48, HP * HP), np.float16)
        for ic in range(IC):
            n_g = 4 * bc + ic
            bp, icp = n_g % 2, n_g // 2
            xpad = np.zeros((A, DSLAB, 52, 52), np.float32)
            lo, hi = max(0, d0), min(D, d0 + DSLAB)
            xpad[:, lo - d0:hi - d0, 1:49, 2:50] = xt[bp, icp, :, lo:hi]
            # win[a, s, kh, kw] = xpad[a, s, kh:kh+50, kw:kw+50]
            win = sliding_window_view(xpad, (HP, HP), axis=(2, 3))
            for t in range(27):
                kd, kh, kw = t // 9, (t % 9) // 3, t % 3
                # [A, DPC, 50, 50] -> [DPC, A, 2500]
                blk = win[:, kd:kd + DPC, kh, kw].transpose(1, 0, 2, 3).reshape(
                    DPC, A, HP * HP).astype(np.float16)
                if t < 24:
                    g, j = t // 8, t % 8
                    xg[:, ic, g, 16 * j:16 * j + 16] = blk
                else:
                    xgc[:, ic, 16 * (t - 24):16 * (t - 24) + 16] = blk
        in_maps.append(dict(xg=xg, xgc=xgc, wg=wg, wgc=wgc, bias=bias,
                            bias8=bias8, masks=masks))
    return in_maps


def assemble_output(results):
    out = np.zeros((B, D, H, W, NC, NA), np.float32)
    for core in range(8):
        bc, dq = core // 4, core % 4
        r = results[core]["out"].astype(np.float32)  # [DPC, OC, 2304]
        r = r.reshape(DPC, NC, NA, H, W).transpose(0, 3, 4, 1, 2)
        out[bc, dq * DPC:(dq + 1) * DPC] = r
    return out


_NC_PROG = None


def _get_prog():
    global _NC_PROG
    if _NC_PROG is None:
        _NC_PROG = build_program()
    return _NC_PROG


def kernel(x, conv_w, b):
    """Full (unsharded) inputs -> full output [2, 24, 48, 48, 8, 16] fp32."""
    from concourse.bass_utils import run_bass_kernel_spmd
    nc = _get_prog()
    in_maps = prep_inputs(x, conv_w, b)
    res = run_bass_kernel_spmd(nc, in_maps, list(range(8)))
    return assemble_output(res.results).astype(np.float32)


def run_traced(x, conv_w, b):
    """Like kernel() but with NTFF tracing; returns (output, BassKernelResults)."""
    try:
        import antenv.axon_hooks as ah
        from trn_agent_boot.trn_boot import _ntff_profile_via_ctypes
        if ah.get_axon_ntff_profile_hook() is None:
            ah.set_axon_ntff_profile_hook(
                _ntff_profile_via_ctypes("/opt/axon/libaxon_pjrt.so"))
    except Exception:
        pass
    from concourse.bass_utils import run_bass_kernel_spmd
    nc = _get_prog()
    in_maps = prep_inputs(x, conv_w, b)
    res = run_bass_kernel_spmd(nc, in_maps, list(range(8)), trace=True)
    return assemble_output(res.results).astype(np.float32), res


# revision 32
# speedup vs baseline: 1.0165x; 1.0165x over previous
"""ConvCapsuleLayer TRN2 kernel v13.

Sharding: 8 cores = B(2) x D-quarters(4); zero cross-core communication.
Structure = v9 (serial blocks, PSUM-resident logits) - measured on HW the
device throttles to ~1.65 concurrently-active engines (activity monitor
util limit 0.5), so wall ~= 0.6 * sum(engine busy); pipelining/offload
don't help (v10 gpsimd offload and v11 3-stage software pipeline both
regressed), only total-engine-time cuts do.

v13 total-work cuts vs v9 (HW: 755us vs v9 771us):
 - weighted_pre: ALL 4 ic route-bcasts consumed straight from PSUM via
   chunked muls (v9 copied ic2/3 to SBUF via ACT then full-width muls:
   3x589 ACT + 660 DVE = 2427ns vs 3x505 = 1515ns per ic-iter).
 - softmax exp in f16 with a -10 logit shift (shift-invariant), so the
   denominator MMs use the f16 esbc mask (f16 MM 420ns vs f32 540ns).
Tried and reverted (regressed on HW): conv copies / t1 on DVE (v12,
812us - DVE became the single-engine bound), gpsimd offload (v10,
844us - Pool shares the DVE SBUF port), 3-stage software pipelining
(v11, 979us - more overlap just means more throttling), wide 4-ic
delta muls (v14, 800us - longer critical chain).
All PE operands fp16 (fp32r/fp32 measured slower); PSUM accumulation
fp32.  Conv kd-MIXED packing: 27 taps baked into contraction as 3x K=128
+ 1x K=48 matmuls per 384-col chunk; shifts/plane-selection host-side.
"""
import sys
import numpy as np

sys.path.insert(0, "/opt/trn_rl_repo")

import concourse.bass as bass
import concourse.mybir as mybir
from concourse import bacc, tile
from contextlib import ExitStack

F32 = mybir.dt.float32
F16 = mybir.dt.float16
AF = mybir.ActivationFunctionType
ALU = mybir.AluOpType

B, D, H, W, IC, A = 2, 24, 48, 48, 4, 16
NC, NA = 8, 16
OC = 128
DPC = 6
DSLAB = DPC + 2
HP = 50
PLANE_POS = H * W      # 2304
CN = 384               # chunk cols (8 h-rows)
NCH = PLANE_POS // CN  # 6 chunks per plane
CROWS = CN // W        # 8
HPL = PLANE_POS // 2   # 1152, half-plane
NCH2 = HPL // CN       # 3 chunks per half-plane


def build_program():
    nc = bacc.Bacc("TRN2", target_bir_lowering=False, debug=False, num_devices=8)
    xg_e = nc.dram_tensor("xg", [DPC, IC, 3, OC, HP * HP], F16, kind="ExternalInput").ap()
    xgc_e = nc.dram_tensor("xgc", [DPC, IC, 48, HP * HP], F16, kind="ExternalInput").ap()
    wg_e = nc.dram_tensor("wg", [3, OC, OC], F16, kind="ExternalInput").ap()
    wgc_e = nc.dram_tensor("wgc", [OC, OC], F16, kind="ExternalInput").ap()
    bias_e = nc.dram_tensor("bias", [OC, 1], F32, kind="ExternalInput").ap()
    bias8_e = nc.dram_tensor("bias8", [OC, 1], F32, kind="ExternalInput").ap()
    masks_e = nc.dram_tensor("masks", [10, OC, OC], F16, kind="ExternalInput").ap()
    out_e = nc.dram_tensor("out", [DPC, OC, PLANE_POS], F16, kind="ExternalOutput").ap()

    with ExitStack() as ctx:
        tc = ctx.enter_context(tile.TileContext(nc))
        cpool = ctx.enter_context(tc.tile_pool(name="const", bufs=1))
        planep = ctx.enter_context(tc.tile_pool(name="planes", bufs=2))
        votesp = ctx.enter_context(tc.tile_pool(name="votes", bufs=2))
        s16 = ctx.enter_context(tc.tile_pool(name="s16", bufs=2))
        s32 = ctx.enter_context(tc.tile_pool(name="s32", bufs=2))
        # conv: ONE double tile (2 banks, chunk pair at 0:384 / 512:896) so the
        # PSUM->SBUF cast copies run 2 chunks per ACT op.  L: double+single
        # (2+1 banks) so softmax exp runs as 2 ACT ops instead of 3.
        ps_conv = ctx.enter_context(tc.tile_pool(name="ps_conv", bufs=1, space="PSUM"))
        ps_L = ctx.enter_context(tc.tile_pool(name="ps_L", bufs=1, space="PSUM"))
        ps_bc = ctx.enter_context(tc.tile_pool(name="ps_bc", bufs=3, space="PSUM"))

        # --- resident constants ---
        wg_s = cpool.tile([OC, 3, OC], F16, tag="wg")
        nc.sync.dma_start(out=wg_s[:], in_=wg_e.rearrange("k p m -> p k m"))
        wgc_s = cpool.tile([OC, OC], F16, tag="wgc")
        nc.sync.dma_start(out=wgc_s[:], in_=wgc_e[:])
        bias_s = cpool.tile([OC, 1], F32, tag="bias")
        nc.sync.dma_start(out=bias_s[:], in_=bias_e[:])
        bias8_s = cpool.tile([OC, 1], F32, tag="bias8")
        nc.sync.dma_start(out=bias8_s[:], in_=bias8_e[:])
        mk_s = cpool.tile([OC, 10, OC], F16, tag="masks")
        nc.sync.dma_start(out=mk_s[:], in_=masks_e.rearrange("k p m -> p k m"))
        nshift = cpool.tile([OC, 1], F32, tag="nshift")
        nc.vector.memset(nshift[:], -10.0)
        esbc = mk_s[:, 0, :]
        ena8bc = mk_s[:, 1, :]
        erbc = [mk_s[:, 2 + i, :] for i in range(IC)]
        edl = [mk_s[:, 6 + i, :] for i in range(IC)]

        def squash_fac(pre, nrm_scale, t1_scale):
            """pre [OC,HPL] f16 -> fac [OC,HPL] f16 (replicated), using
            nb = sum_na (pre)^2 per capsule via ena8bc MM.
            fac = sqrt(nrm_scale*nb) / (1 + t1_scale*nb)."""
            sq = s16.tile([OC, HPL], F16, tag="sq", bufs=3)
            nc.vector.tensor_mul(out=sq[:], in0=pre, in1=pre)
            nbs = []
            for c in range(NCH2):
                nb = ps_bc.tile([OC, CN], F32, tag="bc", name=f"nb{c}")
                nc.tensor.matmul(out=nb[:], lhsT=ena8bc, rhs=sq[:, c * CN:(c + 1) * CN],
                                 start=True, stop=True)
                nbs.append(nb)
            nrm = s16.tile([OC, HPL], F16, tag="nrm")
            for c in range(NCH2):
                nc.scalar.activation(out=nrm[:, c * CN:(c + 1) * CN], in_=nbs[c][:],
                                     func=AF.Sqrt, scale=nrm_scale)
            t1 = s32.tile([OC, HPL], F32, tag="t1")
            for c in range(NCH2):
                nc.scalar.activation(out=t1[:, c * CN:(c + 1) * CN], in_=nbs[c][:],
                                     func=AF.Identity, bias=1.0, scale=t1_scale)
            nc.vector.reciprocal_approx_fast(out=t1[:], in_=t1[:])
            fac = s16.tile([OC, HPL], F16, tag="fac")
            nc.vector.tensor_mul(out=fac[:], in0=nrm[:], in1=t1[:])
            return fac

        def softmax_r(Ls):
            """Ls: list of 3 PSUM logit banks [OC,CN] (L32 layout) ->
            r [OC,HPL] f16 (normalized route, L32 layout).
            exp computed f16 with a -10 logit shift (shift-invariant for
            softmax; measured logits span [-8, 18.5], per-pos max >= 0.3, so
            e <= e^8.5 and the f32 denom stays >= 6e-5 - no over/underflow)."""
            e = s16.tile([OC, HPL], F16, tag="e")
            # Ls[0]/Ls[1] live in one 2-bank tile at +0/+512: one paired exp op
            ein = bass.AP(Ls[0].tensor, Ls[0].offset,
                          [list(Ls[0].ap[0]), [512, 2], [1, CN]])
            eout = bass.AP(e.tensor, e.offset, [list(e.ap[0]), [CN, 2], [1, CN]])
            nc.scalar.activation(out=eout, in_=ein, func=AF.Exp, bias=nshift[:])
            nc.scalar.activation(out=e[:, 2 * CN:3 * CN], in_=Ls[2],
                                 func=AF.Exp, bias=nshift[:])
            rs = s32.tile([OC, HPL], F32, tag="rs")
            for c in range(NCH2):
                sb = ps_bc.tile([OC, CN], F32, tag="bc")
                nc.tensor.matmul(out=sb[:], lhsT=esbc, rhs=e[:, c * CN:(c + 1) * CN],
                                 start=True, stop=True)
                nc.vector.reciprocal_approx_fast(
                    out=rs[:, c * CN:(c + 1) * CN], in_=sb[:])
            r = s16.tile([OC, HPL], F16, tag="r")
            nc.vector.tensor_mul(out=r[:], in0=e[:], in1=rs[:])
            return r

        def weighted_pre(vch, r):
            """pre = sum_ic bcast_ic(r)*votes_ic + bias, [OC,HPL] f16.
            All 4 ic route-bcasts consumed straight from PSUM (chunk muls)."""
            ps = []
            for i in range(IC):
                p = s16.tile([OC, HPL], F16, tag=f"p{i}", name=f"p{i}")
                for c in range(NCH2):
                    rb = ps_bc.tile([OC, CN], F32, tag="bc", name=f"rb{i}{c}")
                    nc.tensor.matmul(out=rb[:], lhsT=erbc[i],
                                     rhs=r[:, c * CN:(c + 1) * CN],
                                     start=True, stop=True)
                    nc.vector.tensor_mul(out=p[:, c * CN:(c + 1) * CN],
                                         in0=vch[i][:, c * CN:(c + 1) * CN],
                                         in1=rb[:])
                ps.append(p)
            a01 = s16.tile([OC, HPL], F16, tag="a01")
            nc.vector.tensor_add(out=a01[:], in0=ps[0][:], in1=ps[1][:])
            a23 = s16.tile([OC, HPL], F16, tag="a23")
            nc.vector.tensor_add(out=a23[:], in0=ps[2][:], in1=ps[3][:])
            pre = s16.tile([OC, HPL], F16, tag="pre")
            # TT (2x) + per-partition bias add (4x) beats the 1x-mode STT
            nc.vector.tensor_add(out=pre[:], in0=a01[:], in1=a23[:])
            nc.vector.tensor_scalar_add(out=pre[:], in0=pre[:], scalar1=bias_s[:])
            return pre

        def delta_accum(vch, act, Ls, first):
            """L_c += per-ic sum_na votes*act (edl MMs, L32 layout)."""
            ds = []
            for i in range(IC):
                d = s16.tile([OC, HPL], F16, tag=f"p{i}", name=f"d{i}")
                nc.vector.tensor_mul(out=d[:], in0=vch[i], in1=act)
                ds.append(d)
            for c in range(NCH2):
                for i in range(IC):
                    nc.tensor.matmul(out=Ls[c], lhsT=edl[i],
                                     rhs=ds[i][:, c * CN:(c + 1) * CN],
                                     start=(first and i == 0), stop=(i == IC - 1))

        # ===================== main loop =====================
        for dp in range(DPC):
            vts = []
            for n in range(IC):
                xt = planep.tile([OC, 3, HP * HP], F16, tag="xt")
                nc.sync.dma_start(out=xt[:], in_=xg_e[dp, n].rearrange("g p m -> p g m"))
                xc = planep.tile([48, HP * HP], F16, tag="xc")
                nc.sync.dma_start(out=xc[:], in_=xgc_e[dp, n])
                v = votesp.tile([OC, PLANE_POS], F16, tag=f"v{n}")
                vts.append(v)
                for c2 in range(NCH // 2):
                    # chunk pair in one 2-bank PSUM tile; one paired cast copy
                    pc2 = ps_conv.tile([OC, 1024], F32, tag="conv", name=f"pc{c2}")
                    for ci in range(2):
                        c = 2 * c2 + ci
                        pc = pc2[:, ci * 512:ci * 512 + CN]
                        off = c * CROWS * HP + 1
                        tA = xt[:]
                        for g in range(3):
                            rhsA = bass.AP(tA.tensor, tA.offset + g * HP * HP + off,
                                           [list(tA.ap[0]), [HP, CROWS], [1, 48]])
                            nc.tensor.matmul(out=pc, lhsT=wg_s[:, g, :],
                                             rhs=rhsA, start=(g == 0), stop=False)
                        tC = xc[0:48, :]
                        rhsC = bass.AP(tC.tensor, tC.offset + off,
                                       [list(tC.ap[0]), [HP, CROWS], [1, 48]])
                        nc.tensor.matmul(out=pc, lhsT=wgc_s[0:48, :],
                                         rhs=rhsC, start=False, stop=True)
                    cin = bass.AP(pc2.tensor, pc2.offset,
                                  [list(pc2.ap[0]), [512, 2], [1, CN]])
                    cout = bass.AP(v.tensor, v.offset + c2 * 2 * CN,
                                   [list(v.ap[0]), [CN, 2], [1, CN]])
                    nc.scalar.activation(out=cout, in_=cin, func=AF.Copy)

            for h in range(2):
                hs = h * HPL
                vch = [vts[n][:, hs:hs + HPL] for n in range(IC)]
                # ---- iter 1: uniform route; P = sum votes + 8*bias ----
                t01 = s16.tile([OC, HPL], F16, tag="t01", bufs=3)
                nc.vector.tensor_add(out=t01[:], in0=vch[0], in1=vch[1])
                t23 = s16.tile([OC, HPL], F16, tag="t23", bufs=3)
                nc.vector.tensor_add(out=t23[:], in0=vch[2], in1=vch[3])
                P = s16.tile([OC, HPL], F16, tag="P")
                nc.vector.tensor_add(out=P[:], in0=t01[:], in1=t23[:])
                nc.vector.tensor_scalar_add(out=P[:], in0=P[:], scalar1=bias8_s[:])
                # pre1_true = P/8: fold 1/64 into t1-scale, (1/8)*(1/8) into
                # nrm-scale (sqrt(nb/4096) = sqrt(n2)/8)
                fac1 = squash_fac(P[:], 1.0 / 4096, 1.0 / 64)
                act1 = s16.tile([OC, HPL], F16, tag="act")
                nc.vector.tensor_mul(out=act1[:], in0=P[:], in1=fac1[:])
                L01 = ps_L.tile([OC, 1024], F32, tag="L2", name="L01")
                L2t = ps_L.tile([OC, CN], F32, tag="L1", name="L2t")
                Ls = [L01[:, 0:CN], L01[:, 512:512 + CN], L2t[:]]
                delta_accum(vch, act1[:], Ls, first=True)
                # ---- iter 2 ----
                r2 = softmax_r(Ls)
                pre2 = weighted_pre(vch, r2)
                fac2 = squash_fac(pre2[:], 1.0, 1.0)
                act2 = s16.tile([OC, HPL], F16, tag="act")
                nc.vector.tensor_mul(out=act2[:], in0=pre2[:], in1=fac2[:])
                delta_accum(vch, act2[:], Ls, first=False)
                # ---- iter 3 ----
                r3 = softmax_r(Ls)
                pre3 = weighted_pre(vch, r3)
                fac3 = squash_fac(pre3[:], 1.0, 1.0)
                o = s16.tile([OC, HPL], F16, tag="o")
                nc.vector.tensor_mul(out=o[:], in0=pre3[:], in1=fac3[:])
                nc.sync.dma_start(out=out_e[dp][:, hs:hs + HPL], in_=o[:])

    nc.compile()
    return nc


# ===================== host side =====================

def prep_inputs(x, conv_w, b):
    x = np.asarray(x, np.float32)
    conv_w = np.asarray(conv_w, np.float32)
    b = np.asarray(b, np.float32)

    wg = np.zeros((3, OC, OC), np.float32)
    wgc = np.zeros((OC, OC), np.float32)
    for t in range(27):
        kd, kh, kw = t // 9, (t % 9) // 3, t % 3
        blk = conv_w[:, :, kd, kh, kw].T  # [16(a), OC]
        if t < 24:
            wg[t // 8, 16 * (t % 8):16 * (t % 8) + 16] = blk
        else:
            wgc[16 * (t - 24):16 * (t - 24) + 16] = blk
    wg = wg.astype(np.float16)
    wgc = wgc.astype(np.float16)

    bias = b[0, 0, 0].reshape(OC, 1).astype(np.float32)
    bias8 = (8.0 * bias).astype(np.float32)

    # masks: [esbc, ena8bc, erbc0-3, edl0-3], each [OC(part) x OC(out)]
    masks = np.zeros((10, OC, OC), np.float32)
    for i in range(IC):
        for n in range(NC):
            for k in range(32):
                masks[0, 32 * i + n, 32 * i + k] = 1.0      # esbc
    for ncp in range(NC):
        for na in range(NA):
            for na2 in range(NA):
                masks[1, 16 * ncp + na, 16 * ncp + na2] = 1.0   # ena8bc
    for i in range(IC):
        for n in range(NC):
            for na in range(NA):
                masks[2 + i, 32 * i + n, 16 * n + na] = 1.0     # erbc_i
    for i in range(IC):
        for ncp in range(NC):
            for na in range(NA):
                for j in range(4):
                    masks[6 + i, 16 * ncp + na, 32 * i + 8 * j + ncp] = 1.0  # edl_i
    masks = masks.astype(np.float16)

    xt = np.transpose(x, (0, 4, 5, 1, 2, 3))  # [B, ICg, A, D, H, W]

    from numpy.lib.stride_tricks import sliding_window_view

    in_maps = []
    for core in range(8):
        bc, dq = core // 4, core % 4
        d0 = dq * DPC - 1
        xg = np.zeros((DPC, IC, 3, OC, HP * HP), np.float16)
        xgc = np.zeros((DPC, IC, 48, HP * HP), np.float16)
        for ic in range(IC):
            n_g = 4 * bc + ic
            bp, icp = n_g % 2, n_g // 2
            xpad = np.zeros((A, DSLAB, 52, 52), np.float32)
            lo, hi = max(0, d0), min(D, d0 + DSLAB)
            xpad[:, lo - d0:hi - d0, 1:49, 2:50] = xt[bp, icp, :, lo:hi]
            # win[a, s, kh, kw] = xpad[a, s, kh:kh+50, kw:kw+50]
            win = sliding_window_view(xpad, (HP, HP), axis=(2, 3))
            for t in range(27):
                kd, kh, kw = t // 9, (t % 9) // 3, t % 3
                # [A, DPC, 50, 50] -> [DPC, A, 2500]
                blk = win[:, kd:kd + DPC, kh, kw].transpose(1, 0, 2, 3).reshape(
                    DPC, A, HP * HP).astype(np.float16)
                if t < 24:
                    g, j = t // 8, t % 8
                    xg[:, ic, g, 16 * j:16 * j + 16] = blk
                else:
                    xgc[:, ic, 16 * (t - 24):16 * (t - 24) + 16] = blk
        in_maps.append(dict(xg=xg, xgc=xgc, wg=wg, wgc=wgc, bias=bias,
                            bias8=bias8, masks=masks))
    return in_maps


def assemble_output(results):
    out = np.zeros((B, D, H, W, NC, NA), np.float32)
    for core in range(8):
        bc, dq = core // 4, core % 4
        r = results[core]["out"].astype(np.float32)  # [DPC, OC, 2304]
        r = r.reshape(DPC, NC, NA, H, W).transpose(0, 3, 4, 1, 2)
        out[bc, dq * DPC:(dq + 1) * DPC] = r
    return out


_NC_PROG = None


def _get_prog():
    global _NC_PROG
    if _NC_PROG is None:
        _NC_PROG = build_program()
    return _NC_PROG


def kernel(x, conv_w, b):
    """Full (unsharded) inputs -> full output [2, 24, 48, 48, 8, 16] fp32."""
    from concourse.bass_utils import run_bass_kernel_spmd
    nc = _get_prog()
    in_maps = prep_inputs(x, conv_w, b)
    res = run_bass_kernel_spmd(nc, in_maps, list(range(8)))
    return assemble_output(res.results).astype(np.float32)


def run_traced(x, conv_w, b):
    """Like kernel() but with NTFF tracing; returns (output, BassKernelResults)."""
    try:
        import antenv.axon_hooks as ah
        from trn_agent_boot.trn_boot import _ntff_profile_via_ctypes
        if ah.get_axon_ntff_profile_hook() is None:
            ah.set_axon_ntff_profile_hook(
                _ntff_profile_via_ctypes("/opt/axon/libaxon_pjrt.so"))
    except Exception:
        pass
    from concourse.bass_utils import run_bass_kernel_spmd
    nc = _get_prog()
    in_maps = prep_inputs(x, conv_w, b)
    res = run_bass_kernel_spmd(nc, in_maps, list(range(8)), trace=True)
    return assemble_output(res.results).astype(np.float32), res


# revision 39
# speedup vs baseline: 1.0281x; 1.0115x over previous
"""ConvCapsuleLayer TRN2 kernel v13.

Sharding: 8 cores = B(2) x D-quarters(4); zero cross-core communication.
Structure = v9 (serial blocks, PSUM-resident logits) - measured on HW the
device throttles to ~1.65 concurrently-active engines (activity monitor
util limit 0.5), so wall ~= 0.6 * sum(engine busy); pipelining/offload
don't help (v10 gpsimd offload and v11 3-stage software pipeline both
regressed), only total-engine-time cuts do.

v13 total-work cuts vs v9 (HW: 755us vs 771us baseline):
 - weighted_pre: ALL 4 ic route-bcasts consumed straight from PSUM via
   chunked muls (v9 copied ic2/3 to SBUF via ACT then full-width muls:
   3x589 ACT + 660 DVE = 2427ns vs 3x505 = 1515ns per ic-iter).
 - softmax exp in f16 with a -10 logit shift (softmax shift-invariant;
   logits measured in [-8, 18.5], per-pos max >= 0.3: no over/underflow),
   so the denom MMs use the f16 esbc mask (420ns vs 540ns fp32 MM).
Regressed on HW and reverted: gpsimd offload (v10 844us; Pool shares
DVE's SBUF port), 3-stage software pipeline (v11 979us; overlap just
triggers more throttle), copies/t1 on DVE (v12 812us; DVE became the
single-engine bound), wide 4-ic delta muls (v14 800us; +1.9us on the
per-iter critical chain), paired 2-bank PSUM copies + exp + STT split
(v15 787us; conv ring-1 serialization, savings below model).
All PE operands fp16 (fp32r/fp32 measured slower); PSUM accumulation
fp32.  Conv kd-MIXED packing: 27 taps baked into contraction as 3x K=128
+ 1x K=48 matmuls per 384-col chunk; shifts/plane-selection host-side.
"""
import sys
import numpy as np

sys.path.insert(0, "/opt/trn_rl_repo")

import concourse.bass as bass
import concourse.mybir as mybir
from concourse import bacc, tile
from contextlib import ExitStack

F32 = mybir.dt.float32
F16 = mybir.dt.float16
AF = mybir.ActivationFunctionType
ALU = mybir.AluOpType

B, D, H, W, IC, A = 2, 24, 48, 48, 4, 16
NC, NA = 8, 16
OC = 128
DPC = 6
DSLAB = DPC + 2
HP = 50
PLANE_POS = H * W      # 2304
CN = 384               # chunk cols (8 h-rows)
NCH = PLANE_POS // CN  # 6 chunks per plane
CROWS = CN // W        # 8
HPL = PLANE_POS // 2   # 1152, half-plane
NCH2 = HPL // CN       # 3 chunks per half-plane


def build_program():
    nc = bacc.Bacc("TRN2", target_bir_lowering=False, debug=False, num_devices=8)
    xg_e = nc.dram_tensor("xg", [DPC, IC, 3, OC, HP * HP], F16, kind="ExternalInput").ap()
    xgc_e = nc.dram_tensor("xgc", [DPC, IC, 48, HP * HP], F16, kind="ExternalInput").ap()
    wg_e = nc.dram_tensor("wg", [3, OC, OC], F16, kind="ExternalInput").ap()
    wgc_e = nc.dram_tensor("wgc", [OC, OC], F16, kind="ExternalInput").ap()
    bias_e = nc.dram_tensor("bias", [OC, 1], F32, kind="ExternalInput").ap()
    bias8_e = nc.dram_tensor("bias8", [OC, 1], F32, kind="ExternalInput").ap()
    masks_e = nc.dram_tensor("masks", [10, OC, OC], F16, kind="ExternalInput").ap()
    out_e = nc.dram_tensor("out", [DPC, OC, PLANE_POS], F16, kind="ExternalOutput").ap()

    with ExitStack() as ctx:
        tc = ctx.enter_context(tile.TileContext(nc))
        cpool = ctx.enter_context(tc.tile_pool(name="const", bufs=1))
        planep = ctx.enter_context(tc.tile_pool(name="planes", bufs=2))
        votesp = ctx.enter_context(tc.tile_pool(name="votes", bufs=2))
        s16 = ctx.enter_context(tc.tile_pool(name="s16", bufs=2))
        s32 = ctx.enter_context(tc.tile_pool(name="s32", bufs=2))
        ps_conv = ctx.enter_context(tc.tile_pool(name="ps_conv", bufs=2, space="PSUM"))
        # logits as one 2-bank tile (chunks 0/1 at +0/+512) + one single bank:
        # same 3 banks and ring behavior, but softmax exp pairs chunks 0/1
        # into a single ACT op (2 ops instead of 3 per softmax).
        ps_L = ctx.enter_context(tc.tile_pool(name="ps_L", bufs=1, space="PSUM"))
        ps_bc = ctx.enter_context(tc.tile_pool(name="ps_bc", bufs=3, space="PSUM"))

        # --- resident constants ---
        wg_s = cpool.tile([OC, 3, OC], F16, tag="wg")
        nc.sync.dma_start(out=wg_s[:], in_=wg_e.rearrange("k p m -> p k m"))
        wgc_s = cpool.tile([OC, OC], F16, tag="wgc")
        nc.sync.dma_start(out=wgc_s[:], in_=wgc_e[:])
        bias_s = cpool.tile([OC, 1], F32, tag="bias")
        nc.sync.dma_start(out=bias_s[:], in_=bias_e[:])
        bias8_s = cpool.tile([OC, 1], F32, tag="bias8")
        nc.sync.dma_start(out=bias8_s[:], in_=bias8_e[:])
        mk_s = cpool.tile([OC, 10, OC], F16, tag="masks")
        nc.sync.dma_start(out=mk_s[:], in_=masks_e.rearrange("k p m -> p k m"))
        nshift = cpool.tile([OC, 1], F32, tag="nshift")
        nc.vector.memset(nshift[:], -10.0)
        esbc = mk_s[:, 0, :]
        ena8bc = mk_s[:, 1, :]
        erbc = [mk_s[:, 2 + i, :] for i in range(IC)]
        edl = [mk_s[:, 6 + i, :] for i in range(IC)]

        def squash_fac(pre, nrm_scale, t1_scale):
            """pre [OC,HPL] f16 -> fac [OC,HPL] f16 (replicated), using
            nb = sum_na (pre)^2 per capsule via ena8bc MM.
            fac = sqrt(nrm_scale*nb) / (1 + t1_scale*nb)."""
            sq = s16.tile([OC, HPL], F16, tag="sq", bufs=3)
            nc.vector.tensor_mul(out=sq[:], in0=pre, in1=pre)
            nbs = []
            for c in range(NCH2):
                nb = ps_bc.tile([OC, CN], F32, tag="bc", name=f"nb{c}")
                nc.tensor.matmul(out=nb[:], lhsT=ena8bc, rhs=sq[:, c * CN:(c + 1) * CN],
                                 start=True, stop=True)
                nbs.append(nb)
            nrm = s16.tile([OC, HPL], F16, tag="nrm")
            for c in range(NCH2):
                nc.scalar.activation(out=nrm[:, c * CN:(c + 1) * CN], in_=nbs[c][:],
                                     func=AF.Sqrt, scale=nrm_scale)
            t1 = s32.tile([OC, HPL], F32, tag="t1")
            for c in range(NCH2):
                nc.scalar.activation(out=t1[:, c * CN:(c + 1) * CN], in_=nbs[c][:],
                                     func=AF.Identity, bias=1.0, scale=t1_scale)
            nc.vector.reciprocal_approx_fast(out=t1[:], in_=t1[:])
            fac = s16.tile([OC, HPL], F16, tag="fac")
            nc.vector.tensor_mul(out=fac[:], in0=nrm[:], in1=t1[:])
            return fac

        def softmax_r(Ls):
            """Ls: list of 3 PSUM logit banks [OC,CN] (L32 layout) ->
            r [OC,HPL] f16 (normalized route, L32 layout).
            exp computed f16 with a -10 logit shift (shift-invariant for
            softmax; measured logits span [-8, 18.5], per-pos max >= 0.3, so
            e <= e^8.5 and the f32 denom stays >= 6e-5 - no over/underflow)."""
            e = s16.tile([OC, HPL], F16, tag="e")
            # Ls[0]/Ls[1] share one 2-bank tile at +0/+512: paired exp op
            ein = bass.AP(Ls[0].tensor, Ls[0].offset,
                          [list(Ls[0].ap[0]), [512, 2], [1, CN]])
            eout = bass.AP(e.tensor, e.offset, [list(e.ap[0]), [CN, 2], [1, CN]])
            nc.scalar.activation(out=eout, in_=ein, func=AF.Exp, bias=nshift[:])
            nc.scalar.activation(out=e[:, 2 * CN:3 * CN], in_=Ls[2],
                                 func=AF.Exp, bias=nshift[:])
            rs = s32.tile([OC, HPL], F32, tag="rs")
            for c in range(NCH2):
                sb = ps_bc.tile([OC, CN], F32, tag="bc")
                nc.tensor.matmul(out=sb[:], lhsT=esbc, rhs=e[:, c * CN:(c + 1) * CN],
                                 start=True, stop=True)
                nc.vector.reciprocal_approx_fast(
                    out=rs[:, c * CN:(c + 1) * CN], in_=sb[:])
            r = s16.tile([OC, HPL], F16, tag="r")
            nc.vector.tensor_mul(out=r[:], in0=e[:], in1=rs[:])
            return r

        def weighted_pre(vch, r):
            """pre = sum_ic bcast_ic(r)*votes_ic + bias, [OC,HPL] f16.
            All 4 ic route-bcasts consumed straight from PSUM (chunk muls)."""
            ps = []
            for i in range(IC):
                p = s16.tile([OC, HPL], F16, tag=f"p{i}", name=f"p{i}")
                for c in range(NCH2):
                    rb = ps_bc.tile([OC, CN], F32, tag="bc", name=f"rb{i}{c}")
                    nc.tensor.matmul(out=rb[:], lhsT=erbc[i],
                                     rhs=r[:, c * CN:(c + 1) * CN],
                                     start=True, stop=True)
                    nc.vector.tensor_mul(out=p[:, c * CN:(c + 1) * CN],
                                         in0=vch[i][:, c * CN:(c + 1) * CN],
                                         in1=rb[:])
                ps.append(p)
            a01 = s16.tile([OC, HPL], F16, tag="a01")
            nc.vector.tensor_add(out=a01[:], in0=ps[0][:], in1=ps[1][:])
            a23 = s16.tile([OC, HPL], F16, tag="a23")
            nc.vector.tensor_add(out=a23[:], in0=ps[2][:], in1=ps[3][:])
            pre = s16.tile([OC, HPL], F16, tag="pre")
            # TT (2x mode) + per-partition bias add (4x) beats the 1x STT
            nc.vector.tensor_add(out=pre[:], in0=a01[:], in1=a23[:])
            nc.vector.tensor_scalar_add(out=pre[:], in0=pre[:], scalar1=bias_s[:])
            return pre

        def delta_accum(vch, act, Ls, first):
            """L_c += per-ic sum_na votes*act (edl MMs, L32 layout)."""
            ds = []
            for i in range(IC):
                d = s16.tile([OC, HPL], F16, tag=f"p{i}", name=f"d{i}")
                nc.vector.tensor_mul(out=d[:], in0=vch[i], in1=act)
                ds.append(d)
            for c in range(NCH2):
                for i in range(IC):
                    nc.tensor.matmul(out=Ls[c], lhsT=edl[i],
                                     rhs=ds[i][:, c * CN:(c + 1) * CN],
                                     start=(first and i == 0), stop=(i == IC - 1))

        # ===================== main loop =====================
        for dp in range(DPC):
            vts = []
            for n in range(IC):
                xt = planep.tile([OC, 3, HP * HP], F16, tag="xt")
                nc.sync.dma_start(out=xt[:], in_=xg_e[dp, n].rearrange("g p m -> p g m"))
                xc = planep.tile([48, HP * HP], F16, tag="xc")
                nc.sync.dma_start(out=xc[:], in_=xgc_e[dp, n])
                v = votesp.tile([OC, PLANE_POS], F16, tag=f"v{n}")
                vts.append(v)
                for c in range(NCH):
                    h0 = c * CROWS
                    pc = ps_conv.tile([OC, CN], F32, tag="conv")
                    off = h0 * HP + 1
                    tA = xt[:]
                    for g in range(3):
                        rhsA = bass.AP(tA.tensor, tA.offset + g * HP * HP + off,
                                       [list(tA.ap[0]), [HP, CROWS], [1, 48]])
                        nc.tensor.matmul(out=pc[:], lhsT=wg_s[:, g, :],
                                         rhs=rhsA, start=(g == 0), stop=False)
                    tC = xc[0:48, :]
                    rhsC = bass.AP(tC.tensor, tC.offset + off,
                                   [list(tC.ap[0]), [HP, CROWS], [1, 48]])
                    nc.tensor.matmul(out=pc[:], lhsT=wgc_s[0:48, :],
                                     rhs=rhsC, start=False, stop=True)
                    nc.scalar.copy(out=v[:, c * CN:(c + 1) * CN], in_=pc[:])

            for h in range(2):
                hs = h * HPL
                vch = [vts[n][:, hs:hs + HPL] for n in range(IC)]
                # ---- iter 1: uniform route; P = sum votes + 8*bias ----
                t01 = s16.tile([OC, HPL], F16, tag="t01", bufs=3)
                nc.vector.tensor_add(out=t01[:], in0=vch[0], in1=vch[1])
                t23 = s16.tile([OC, HPL], F16, tag="t23", bufs=3)
                nc.vector.tensor_add(out=t23[:], in0=vch[2], in1=vch[3])
                P = s16.tile([OC, HPL], F16, tag="P")
                nc.vector.tensor_add(out=P[:], in0=t01[:], in1=t23[:])
                nc.vector.tensor_scalar_add(out=P[:], in0=P[:], scalar1=bias8_s[:])
                # pre1_true = P/8: fold 1/64 into t1-scale, (1/8)*(1/8) into
                # nrm-scale (sqrt(nb/4096) = sqrt(n2)/8)
                fac1 = squash_fac(P[:], 1.0 / 4096, 1.0 / 64)
                act1 = s16.tile([OC, HPL], F16, tag="act")
                nc.vector.tensor_mul(out=act1[:], in0=P[:], in1=fac1[:])
                L01 = ps_L.tile([OC, 1024], F32, tag="L2", name="L01")
                L2t = ps_L.tile([OC, CN], F32, tag="L1", name="L2t")
                Ls = [L01[:, 0:CN], L01[:, 512:512 + CN], L2t[:]]
                delta_accum(vch, act1[:], Ls, first=True)
                # ---- iter 2 ----
                r2 = softmax_r(Ls)
                pre2 = weighted_pre(vch, r2)
                fac2 = squash_fac(pre2[:], 1.0, 1.0)
                act2 = s16.tile([OC, HPL], F16, tag="act")
                nc.vector.tensor_mul(out=act2[:], in0=pre2[:], in1=fac2[:])
                delta_accum(vch, act2[:], Ls, first=False)
                # ---- iter 3 ----
                r3 = softmax_r(Ls)
                pre3 = weighted_pre(vch, r3)
                fac3 = squash_fac(pre3[:], 1.0, 1.0)
                o = s16.tile([OC, HPL], F16, tag="o")
                nc.vector.tensor_mul(out=o[:], in0=pre3[:], in1=fac3[:])
                nc.sync.dma_start(out=out_e[dp][:, hs:hs + HPL], in_=o[:])

    nc.compile()
    return nc


# ===================== host side =====================

def prep_inputs(x, conv_w, b):
    x = np.asarray(x, np.float32)
    conv_w = np.asarray(conv_w, np.float32)
    b = np.asarray(b, np.float32)

    wg = np.zeros((3, OC, OC), np.float32)
    wgc = np.zeros((OC, OC), np.float32)
    for t in range(27):
        kd, kh, kw = t // 9, (t % 9) // 3, t % 3
        blk = conv_w[:, :, kd, kh, kw].T  # [16(a), OC]
        if t < 24:
            wg[t // 8, 16 * (t % 8):16 * (t % 8) + 16] = blk
        else:
            wgc[16 * (t - 24):16 * (t - 24) + 16] = blk
    wg = wg.astype(np.float16)
    wgc = wgc.astype(np.float16)

    bias = b[0, 0, 0].reshape(OC, 1).astype(np.float32)
    bias8 = (8.0 * bias).astype(np.float32)

    # masks: [esbc, ena8bc, erbc0-3, edl0-3], each [OC(part) x OC(out)]
    masks = np.zeros((10, OC, OC), np.float32)
    for i in range(IC):
        for n in range(NC):
            for k in range(32):
                masks[0, 32 * i + n, 32 * i + k] = 1.0      # esbc
    for ncp in range(NC):
        for na in range(NA):
            for na2 in range(NA):
                masks[1, 16 * ncp + na, 16 * ncp + na2] = 1.0   # ena8bc
    for i in range(IC):
        for n in range(NC):
            for na in range(NA):
                masks[2 + i, 32 * i + n, 16 * n + na] = 1.0     # erbc_i
    for i in range(IC):
        for ncp in range(NC):
            for na in range(NA):
                for j in range(4):
                    masks[6 + i, 16 * ncp + na, 32 * i + 8 * j + ncp] = 1.0  # edl_i
    masks = masks.astype(np.float16)

    xt = np.transpose(x, (0, 4, 5, 1, 2, 3))  # [B, ICg, A, D, H, W]

    from numpy.lib.stride_tricks import sliding_window_view

    in_maps = []
    for core in range(8):
        bc, dq = core // 4, core % 4
        d0 = dq * DPC - 1
        xg = np.zeros((DPC, IC, 3, OC, HP * HP), np.float16)
        xgc = np.zeros((DPC, IC, 48, HP * HP), np.float16)
        for ic in range(IC):
            n_g = 4 * bc + ic
            bp, icp = n_g % 2, n_g // 2
            xpad = np.zeros((A, DSLAB, 52, 52), np.float32)
            lo, hi = max(0, d0), min(D, d0 + DSLAB)
            xpad[:, lo - d0:hi - d0, 1:49, 2:50] = xt[bp, icp, :, lo:hi]
            # win[a, s, kh, kw] = xpad[a, s, kh:kh+50, kw:kw+50]
            win = sliding_window_view(xpad, (HP, HP), axis=(2, 3))
            for t in range(27):
                kd, kh, kw = t // 9, (t % 9) // 3, t % 3
                # [A, DPC, 50, 50] -> [DPC, A, 2500]
                blk = win[:, kd:kd + DPC, kh, kw].transpose(1, 0, 2, 3).reshape(
                    DPC, A, HP * HP).astype(np.float16)
                if t < 24:
                    g, j = t // 8, t % 8
                    xg[:, ic, g, 16 * j:16 * j + 16] = blk
                else:
                    xgc[:, ic, 16 * (t - 24):16 * (t - 24) + 16] = blk
        in_maps.append(dict(xg=xg, xgc=xgc, wg=wg, wgc=wgc, bias=bias,
                            bias8=bias8, masks=masks))
    return in_maps


def assemble_output(results):
    out = np.zeros((B, D, H, W, NC, NA), np.float32)
    for core in range(8):
        bc, dq = core // 4, core % 4
        r = results[core]["out"].astype(np.float32)  # [DPC, OC, 2304]
        r = r.reshape(DPC, NC, NA, H, W).transpose(0, 3, 4, 1, 2)
        out[bc, dq * DPC:(dq + 1) * DPC] = r
    return out


_NC_PROG = None


def _get_prog():
    global _NC_PROG
    if _NC_PROG is None:
        _NC_PROG = build_program()
    return _NC_PROG


def kernel(x, conv_w, b):
    """Full (unsharded) inputs -> full output [2, 24, 48, 48, 8, 16] fp32."""
    from concourse.bass_utils import run_bass_kernel_spmd
    nc = _get_prog()
    in_maps = prep_inputs(x, conv_w, b)
    res = run_bass_kernel_spmd(nc, in_maps, list(range(8)))
    return assemble_output(res.results).astype(np.float32)


def run_traced(x, conv_w, b):
    """Like kernel() but with NTFF tracing; returns (output, BassKernelResults)."""
    try:
        import antenv.axon_hooks as ah
        from trn_agent_boot.trn_boot import _ntff_profile_via_ctypes
        if ah.get_axon_ntff_profile_hook() is None:
            ah.set_axon_ntff_profile_hook(
                _ntff_profile_via_ctypes("/opt/axon/libaxon_pjrt.so"))
    except Exception:
        pass
    from concourse.bass_utils import run_bass_kernel_spmd
    nc = _get_prog()
    in_maps = prep_inputs(x, conv_w, b)
    res = run_bass_kernel_spmd(nc, in_maps, list(range(8)), trace=True)
    return assemble_output(res.results).astype(np.float32), res
